# revision 27
# baseline (speedup 1.0000x reference)
"""Trainium2 Bass kernel for the MinimalRNN problem.

Strategy (data-parallel over batch, per the sharding hint):
  - 8 NeuronCores, each owns a batch slice of 8 (B=64 total).
  - Everything on-chip uses a transposed layout: hidden/feature dim on
    SBUF partitions, (seq*batch) on the free dim.  With that layout the
    sequential recurrence  h_t = tanh(xh_t + W_hh @ h_{t-1})  maps to
    weight-stationary matmuls (lhsT = W_hh^T tiles, rhs = h^T) whose
    output is already transposed for the next step - no transposes in
    the inner loop.
  - fp16 weights + activations (abs-max rel err ~4e-4 over 512 steps),
    fp32 PSUM accumulation.
  - xh^T is precomputed for all timesteps (one big matmul), the tanh
    output overwrites the consumed xh slot in place, and the final
    projection reads the same buffer as hs^T.

Host side only slices/transposes/casts numpy arrays; all FLOPs run on
the NeuronCores.
"""

import sys

sys.path.insert(0, "/opt/trn_rl_repo")

from contextlib import ExitStack

import numpy as np

import concourse.bass as bass
import concourse.bacc as bacc
import concourse.mybir as mybir
from concourse.tile import TileContext

SEQ, B, IN, HID, OUT = 512, 64, 512, 1024, 512
NCORES = 8
BC = B // NCORES            # batch per core = 8
COLS = SEQ * BC             # free-dim length of the big buffers = 4096
F16 = mybir.dt.float16
F32 = mybir.dt.float32
KX = IN // 128              # 4  k-tiles for the input projection
KH = HID // 128             # 8  k-tiles / m-tiles for the hidden dim
MO = OUT // 128             # 4  m-tiles for the output projection
NCH = COLS // 512           # 8  512-wide column chunks
TANH = mybir.ActivationFunctionType.Tanh


def build_nc(seq: int = SEQ) -> bass.Bass:
    cols = seq * BC
    CH = min(512, cols)          # psum chunk width (512 except tiny smoke tests)
    nch = cols // CH
    nc = bacc.Bacc(
        "TRN2",
        target_bir_lowering=False,
        debug=False,
        num_devices=NCORES,
    )

    xT = nc.dram_tensor("xT", [IN, cols], F16, kind="ExternalInput")
    w_xhT = nc.dram_tensor("w_xhT", [IN, HID], F16, kind="ExternalInput")
    w_hhT = nc.dram_tensor("w_hhT", [HID, HID], F16, kind="ExternalInput")
    w_outT = nc.dram_tensor("w_outT", [HID, OUT], F16, kind="ExternalInput")
    bias_h = nc.dram_tensor("bias_h", [128, KH], F32, kind="ExternalInput")
    bias_o = nc.dram_tensor("bias_o", [128, MO], F32, kind="ExternalInput")
    outT = nc.dram_tensor("outT", [OUT, cols], F32, kind="ExternalOutput")
    hT_last = nc.dram_tensor("hT_last", [HID, BC], F16, kind="ExternalOutput")

    with TileContext(nc) as tc, ExitStack() as ctx:
        const = ctx.enter_context(tc.tile_pool(name="const", bufs=1))
        seqp = ctx.enter_context(tc.tile_pool(name="seqbuf", bufs=1))

        # DMA order: the input projection's operands (W_xh, x, bias) first
        # so the PE can start ~10us earlier; W_hh/W_out are not needed until
        # the recurrence / output projection.
        wxh_sb = []
        for k in range(KX):
            t = const.tile([128, HID], F16, name=f"wxh{k}")
            nc.sync.dma_start(out=t, in_=w_xhT[k * 128:(k + 1) * 128, :])
            wxh_sb.append(t)
        bh = const.tile([128, KH], F32, name="bh")
        nc.sync.dma_start(out=bh, in_=bias_h[:, :])
        xsb = []
        for k in range(KX):
            t = const.tile([128, cols], F16, name=f"x{k}")
            nc.sync.dma_start(out=t, in_=xT[k * 128:(k + 1) * 128, :])
            xsb.append(t)
        whh_sb = []
        for k in range(KH):
            t = const.tile([128, HID], F16, name=f"whh{k}")
            nc.sync.dma_start(out=t, in_=w_hhT[k * 128:(k + 1) * 128, :])
            whh_sb.append(t)
        wout_sb = []
        for k in range(KH):
            t = const.tile([128, OUT], F16, name=f"wout{k}")
            nc.sync.dma_start(out=t, in_=w_outT[k * 128:(k + 1) * 128, :])
            wout_sb.append(t)
        bo = const.tile([128, MO], F32, name="bo")
        nc.sync.dma_start(out=bo, in_=bias_o[:, :])

        # xh^T buffer; overwritten in place by h^T as the recurrence runs.
        # One tensor, logically (128, KH, cols): block m holds hidden rows
        # [128m, 128(m+1)).
        sq_all = seqp.tile([128, KH * cols], F16, name="seqall")
        sq = [sq_all[:, m * cols:(m + 1) * cols] for m in range(KH)]
        sqr = sq_all.rearrange("p (m c) -> p m c", m=KH)

        # fp16 identity, used to accumulate xh into PSUM via the PE so the
        # per-step dependency chain is psum -> tanh (no DVE hop).
        ident = const.tile([128, 128], F16, name="ident")
        idram = nc.dram_tensor("ident_dram", [128, 128], F16, kind="ExternalInput")
        nc.sync.dma_start(out=ident, in_=idram[:, :])

        # ---- input projection: sq[m] = W_xh @ x^T + (b_xh + b_hh) ----
        with tc.tile_pool(name="psum_in", bufs=4, space="PSUM") as psum_in:
            for c in range(nch):
                for m in range(KH):
                    ps = psum_in.tile([128, CH], F32, name="ps_in")
                    for k in range(KX):
                        nc.tensor.matmul(
                            ps,
                            lhsT=wxh_sb[k][:, m * 128:(m + 1) * 128],
                            rhs=xsb[k][:, c * CH:(c + 1) * CH],
                            start=(k == 0),
                            stop=(k == KX - 1),
                        )
                    nc.vector.tensor_scalar_add(
                        sq[m][:, c * CH:(c + 1) * CH], ps, bh[:, m:m + 1]
                    )

        # ---- recurrence ----
        with tc.tile_pool(name="psum_h", bufs=8, space="PSUM") as psum_h, \
                tc.tile_pool(name="tmp", bufs=8) as tmpp:
            # t = 0: h_1 = tanh(xh_0)  (h_0 = 0).  Not in-place: Tile's
            # dependency tracker mis-handles out==in on one instruction.
            for m in range(KH):
                tmp0 = tmpp.tile([128, BC], F32, name="tmp_h")
                nc.vector.tensor_copy(tmp0, sq[m][:, 0:BC])
                nc.scalar.activation(sq[m][:, 0:BC], tmp0, TANH)
            # m-major over groups; each PAIR of groups (2m, 2m+1) shares one
            # PSUM bank (128x16): the pair's first matmul start=True clears
            # the bank, and has_written turns each element's first write into
            # an overwrite, so the second group's accumulation is correct
            # with start=False.  One tanh ACT per pair (ACT cost is ~fixed
            # per instruction, so 4 wide ACTs beat 8 narrow ones) keeps the
            # scalar engine at ~40% load -> its ticks fire promptly and the
            # PE never waits long at step boundaries.  xh_t enters PSUM via
            # per-group identity matmuls (no DVE in the chain).
            # Two-phase k-order: phase 1 streams k<4 for every group
            # (consuming h-tiles tanh'd early in the previous step), the
            # identity matmuls seed xh, then phase 2 finishes k>=4 per pair
            # and fires its tanh immediately - so each of the 4 ACT ticks is
            # produced as early and consumed as late as possible.
            KHALF = KH // 2
            # Bank layout per step: blocks 0 and 1 (the first consumed by
            # the next step's phase 1) get their OWN banks and narrow tanh
            # ACTs, so their ticks fire ~0.5us earlier (no wait for a bank
            # mate, no whole-pair serialization).  Blocks 2..7 share banks
            # in pairs with one wide ACT each, keeping total ACT count at 5.
            # groups[i] = (list_of_blocks, psum_width)
            groups = [([0], BC), ([1], BC)] + [
                ([2 * mp, 2 * mp + 1], 2 * BC) for mp in range(1, KH // 2)
            ]
            for t in range(1, seq):
                prev = slice((t - 1) * BC, t * BC)
                cur = slice(t * BC, (t + 1) * BC)
                pss = [psum_h.tile([128, w], F32, name="ps_h",
                                    padded_shape=[128, 2 * BC])
                       for _, w in groups]
                for k in range(KHALF):
                    for gi, (blocks, _) in enumerate(groups):
                        for hi, m in enumerate(blocks):
                            nc.tensor.matmul(
                                pss[gi][:, hi * BC:(hi + 1) * BC],
                                lhsT=whh_sb[k][:, m * 128:(m + 1) * 128],
                                rhs=sq[k][:, prev],
                                start=(k == 0 and hi == 0),
                                stop=False,
                                skip_group_check=True,
                            )
                for gi, (blocks, _) in enumerate(groups):
                    nc.tensor.matmul(
                        pss[gi],
                        lhsT=ident,
                        rhs=sqr[:, blocks[0]:blocks[-1] + 1, cur],
                        start=False, stop=False, skip_group_check=True,
                    )
                for gi, (blocks, _) in enumerate(groups):
                    for hi, m in enumerate(blocks):
                        for k in range(KHALF, KH):
                            nc.tensor.matmul(
                                pss[gi][:, hi * BC:(hi + 1) * BC],
                                lhsT=whh_sb[k][:, m * 128:(m + 1) * 128],
                                rhs=sq[k][:, prev],
                                start=False,
                                stop=(gi == len(groups) - 1
                                      and hi == len(blocks) - 1
                                      and k == KH - 1),
                                skip_group_check=True,
                            )
                    nc.scalar.activation(
                        sqr[:, blocks[0]:blocks[-1] + 1, cur],
                        pss[gi].rearrange("p (g b) -> p g b", g=len(blocks)),
                        TANH,
                    )

        # ---- output projection: outT = W_out @ hs^T + b_out ----
        with tc.tile_pool(name="psum_o", bufs=4, space="PSUM") as psum_o, \
                tc.tile_pool(name="ostg", bufs=4) as ostg:
            for mo in range(MO):
                for c in range(nch):
                    ps = psum_o.tile([128, CH], F32, name="ps_o")
                    for k in range(KH):
                        nc.tensor.matmul(
                            ps,
                            lhsT=wout_sb[k][:, mo * 128:(mo + 1) * 128],
                            rhs=sq[k][:, c * CH:(c + 1) * CH],
                            start=(k == 0),
                            stop=(k == KH - 1),
                        )
                    st = ostg.tile([128, CH], F32, name="ostg")
                    nc.vector.tensor_scalar_add(st, ps, bo[:, mo:mo + 1])
                    nc.sync.dma_start(
                        out=outT[mo * 128:(mo + 1) * 128, c * CH:(c + 1) * CH],
                        in_=st,
                    )
            for k in range(KH):
                nc.sync.dma_start(
                    out=hT_last[k * 128:(k + 1) * 128, :],
                    in_=sq[k][:, (seq - 1) * BC:seq * BC],
                )
    nc.compile()
    return nc


def make_in_maps(x, W_xh, b_xh, W_hh, b_hh, W_out, b_out, seq: int = SEQ):
    """Host-side shard + layout prep. Returns one input dict per core."""
    x = np.asarray(x, np.float32)
    w_xhT = np.ascontiguousarray(np.asarray(W_xh, np.float32).T.astype(np.float16))
    w_hhT = np.ascontiguousarray(np.asarray(W_hh, np.float32).T.astype(np.float16))
    w_outT = np.ascontiguousarray(np.asarray(W_out, np.float32).T.astype(np.float16))
    bias_h = np.ascontiguousarray(
        (np.asarray(b_xh, np.float32) + np.asarray(b_hh, np.float32))
        .reshape(KH, 128).T
    )
    bias_o = np.ascontiguousarray(np.asarray(b_out, np.float32).reshape(MO, 128).T)

    in_maps = []
    for c in range(NCORES):
        xs = x[:seq, c * BC:(c + 1) * BC, :]          # (seq, BC, IN)
        xTc = np.ascontiguousarray(
            xs.transpose(2, 0, 1).reshape(IN, seq * BC).astype(np.float16)
        )
        in_maps.append({
            "ident_dram": np.eye(128, dtype=np.float16),
            "xT": xTc,
            "w_xhT": w_xhT,
            "w_hhT": w_hhT,
            "w_outT": w_outT,
            "bias_h": bias_h,
            "bias_o": bias_o,
        })
    return in_maps


def assemble_output(results, seq: int = SEQ):
    outputs = np.empty((seq, B, OUT), np.float32)
    h_last = np.empty((B, HID), np.float32)
    for c in range(NCORES):
        oT = np.asarray(results[c]["outT"])           # (OUT, seq*BC) f32
        outputs[:, c * BC:(c + 1) * BC, :] = (
            oT.reshape(OUT, seq, BC).transpose(1, 2, 0)
        )
        h_last[c * BC:(c + 1) * BC, :] = (
            np.asarray(results[c]["hT_last"]).astype(np.float32).T
        )
    return outputs, h_last


def kernel(x, W_xh, b_xh, W_hh, b_hh, W_out, b_out):
    from concourse.bass_utils import run_bass_kernel_spmd

    nc = build_nc(SEQ)
    in_maps = make_in_maps(x, W_xh, b_xh, W_hh, b_hh, W_out, b_out, SEQ)
    res = run_bass_kernel_spmd(nc, in_maps, list(range(NCORES)))
    return assemble_output(res.results, SEQ)


# revision 29
# speedup vs baseline: 1.0174x; 1.0174x over previous
"""Trainium2 Bass kernel for the MinimalRNN problem.

Strategy (data-parallel over batch, per the sharding hint):
  - 8 NeuronCores, each owns a batch slice of 8 (B=64 total).
  - Everything on-chip uses a transposed layout: hidden/feature dim on
    SBUF partitions, (seq*batch) on the free dim.  With that layout the
    sequential recurrence  h_t = tanh(xh_t + W_hh @ h_{t-1})  maps to
    weight-stationary matmuls (lhsT = W_hh^T tiles, rhs = h^T) whose
    output is already transposed for the next step - no transposes in
    the inner loop.
  - fp16 weights + activations (abs-max rel err ~4e-4 over 512 steps),
    fp32 PSUM accumulation.
  - xh^T is precomputed for all timesteps (one big matmul), the tanh
    output overwrites the consumed xh slot in place, and the final
    projection reads the same buffer as hs^T.

Host side only slices/transposes/casts numpy arrays; all FLOPs run on
the NeuronCores.
"""

import sys

sys.path.insert(0, "/opt/trn_rl_repo")

from contextlib import ExitStack

import numpy as np

import concourse.bass as bass
import concourse.bacc as bacc
import concourse.mybir as mybir
from concourse.tile import TileContext

SEQ, B, IN, HID, OUT = 512, 64, 512, 1024, 512
NCORES = 8
BC = B // NCORES            # batch per core = 8
COLS = SEQ * BC             # free-dim length of the big buffers = 4096
F16 = mybir.dt.float16
F32 = mybir.dt.float32
KX = IN // 128              # 4  k-tiles for the input projection
KH = HID // 128             # 8  k-tiles / m-tiles for the hidden dim
MO = OUT // 128             # 4  m-tiles for the output projection
NCH = COLS // 512           # 8  512-wide column chunks
TANH = mybir.ActivationFunctionType.Tanh


def build_nc(seq: int = SEQ) -> bass.Bass:
    cols = seq * BC
    CH = min(512, cols)          # psum chunk width (512 except tiny smoke tests)
    nch = cols // CH
    nc = bacc.Bacc(
        "TRN2",
        target_bir_lowering=False,
        debug=False,
        num_devices=NCORES,
    )

    xT = nc.dram_tensor("xT", [IN, cols], F16, kind="ExternalInput")
    w_xhT = nc.dram_tensor("w_xhT", [IN, HID], F16, kind="ExternalInput")
    w_hhT = nc.dram_tensor("w_hhT", [HID, HID], F16, kind="ExternalInput")
    w_outT = nc.dram_tensor("w_outT", [HID, OUT], F16, kind="ExternalInput")
    bias_h = nc.dram_tensor("bias_h", [128, KH], F32, kind="ExternalInput")
    bias_o = nc.dram_tensor("bias_o", [128, MO], F32, kind="ExternalInput")
    outT = nc.dram_tensor("outT", [OUT, cols], F32, kind="ExternalOutput")
    hT_last = nc.dram_tensor("hT_last", [HID, BC], F16, kind="ExternalOutput")

    with TileContext(nc) as tc, ExitStack() as ctx:
        const = ctx.enter_context(tc.tile_pool(name="const", bufs=1))
        seqp = ctx.enter_context(tc.tile_pool(name="seqbuf", bufs=1))

        # DMA order: the input projection's operands (W_xh, x, bias) first
        # so the PE can start ~10us earlier; W_hh/W_out are not needed until
        # the recurrence / output projection.
        wxh_sb = []
        for k in range(KX):
            t = const.tile([128, HID], F16, name=f"wxh{k}")
            nc.sync.dma_start(out=t, in_=w_xhT[k * 128:(k + 1) * 128, :])
            wxh_sb.append(t)
        bh = const.tile([128, KH], F32, name="bh")
        nc.sync.dma_start(out=bh, in_=bias_h[:, :])
        # x arrives in two waves: the first CH columns (all the input
        # projection's chunk 0 needs) land right after W_xh so the PE can
        # start ~10us earlier; the remainder streams behind it.
        xsb = []
        for k in range(KX):
            t = const.tile([128, cols], F16, name=f"x{k}")
            nc.sync.dma_start(out=t[:, 0:CH], in_=xT[k * 128:(k + 1) * 128, 0:CH])
            xsb.append(t)
        for k in range(KX):
            if cols > CH:
                nc.sync.dma_start(
                    out=xsb[k][:, CH:cols], in_=xT[k * 128:(k + 1) * 128, CH:cols]
                )
        whh_sb = []
        for k in range(KH):
            t = const.tile([128, HID], F16, name=f"whh{k}")
            nc.sync.dma_start(out=t, in_=w_hhT[k * 128:(k + 1) * 128, :])
            whh_sb.append(t)
        wout_sb = []
        for k in range(KH):
            t = const.tile([128, OUT], F16, name=f"wout{k}")
            nc.sync.dma_start(out=t, in_=w_outT[k * 128:(k + 1) * 128, :])
            wout_sb.append(t)
        bo = const.tile([128, MO], F32, name="bo")
        nc.sync.dma_start(out=bo, in_=bias_o[:, :])

        # xh^T buffer; overwritten in place by h^T as the recurrence runs.
        # One tensor, logically (128, KH, cols): block m holds hidden rows
        # [128m, 128(m+1)).
        sq_all = seqp.tile([128, KH * cols], F16, name="seqall")
        sq = [sq_all[:, m * cols:(m + 1) * cols] for m in range(KH)]
        sqr = sq_all.rearrange("p (m c) -> p m c", m=KH)

        # fp16 identity, used to accumulate xh into PSUM via the PE so the
        # per-step dependency chain is psum -> tanh (no DVE hop).
        ident = const.tile([128, 128], F16, name="ident")
        idram = nc.dram_tensor("ident_dram", [128, 128], F16, kind="ExternalInput")
        nc.sync.dma_start(out=ident, in_=idram[:, :])

        # ---- input projection: sq[m] = W_xh @ x^T + (b_xh + b_hh) ----
        with tc.tile_pool(name="psum_in", bufs=4, space="PSUM") as psum_in:
            for c in range(nch):
                for m in range(KH):
                    ps = psum_in.tile([128, CH], F32, name="ps_in")
                    for k in range(KX):
                        nc.tensor.matmul(
                            ps,
                            lhsT=wxh_sb[k][:, m * 128:(m + 1) * 128],
                            rhs=xsb[k][:, c * CH:(c + 1) * CH],
                            start=(k == 0),
                            stop=(k == KX - 1),
                        )
                    nc.vector.tensor_scalar_add(
                        sq[m][:, c * CH:(c + 1) * CH], ps, bh[:, m:m + 1]
                    )

        # ---- recurrence ----
        with tc.tile_pool(name="psum_h", bufs=8, space="PSUM") as psum_h, \
                tc.tile_pool(name="tmp", bufs=8) as tmpp:
            # t = 0: h_1 = tanh(xh_0)  (h_0 = 0).  Not in-place: Tile's
            # dependency tracker mis-handles out==in on one instruction.
            for m in range(KH):
                tmp0 = tmpp.tile([128, BC], F32, name="tmp_h")
                nc.vector.tensor_copy(tmp0, sq[m][:, 0:BC])
                nc.scalar.activation(sq[m][:, 0:BC], tmp0, TANH)
            # m-major over groups; each PAIR of groups (2m, 2m+1) shares one
            # PSUM bank (128x16): the pair's first matmul start=True clears
            # the bank, and has_written turns each element's first write into
            # an overwrite, so the second group's accumulation is correct
            # with start=False.  One tanh ACT per pair (ACT cost is ~fixed
            # per instruction, so 4 wide ACTs beat 8 narrow ones) keeps the
            # scalar engine at ~40% load -> its ticks fire promptly and the
            # PE never waits long at step boundaries.  xh_t enters PSUM via
            # per-group identity matmuls (no DVE in the chain).
            # Two-phase k-order: phase 1 streams k<4 for every group
            # (consuming h-tiles tanh'd early in the previous step), the
            # identity matmuls seed xh, then phase 2 finishes k>=4 per pair
            # and fires its tanh immediately - so each of the 4 ACT ticks is
            # produced as early and consumed as late as possible.
            KHALF = KH // 2
            for t in range(1, seq):
                prev = slice((t - 1) * BC, t * BC)
                cur = slice(t * BC, (t + 1) * BC)
                pss = [psum_h.tile([128, 2 * BC], F32, name="ps_h")
                       for _ in range(KH // 2)]
                for k in range(KHALF):
                    for mp in range(KH // 2):
                        for half in range(2):
                            m = 2 * mp + half
                            nc.tensor.matmul(
                                pss[mp][:, half * BC:(half + 1) * BC],
                                lhsT=whh_sb[k][:, m * 128:(m + 1) * 128],
                                rhs=sq[k][:, prev],
                                start=(k == 0 and half == 0),
                                stop=False,
                                skip_group_check=True,
                            )
                for mp in range(KH // 2):
                    nc.tensor.matmul(
                        pss[mp], lhsT=ident, rhs=sqr[:, 2 * mp:2 * mp + 2, cur],
                        start=False, stop=False, skip_group_check=True,
                    )
                for mp in range(KH // 2):
                    for half in range(2):
                        m = 2 * mp + half
                        for k in range(KHALF, KH):
                            nc.tensor.matmul(
                                pss[mp][:, half * BC:(half + 1) * BC],
                                lhsT=whh_sb[k][:, m * 128:(m + 1) * 128],
                                rhs=sq[k][:, prev],
                                start=False,
                                stop=(half == 1 and k == KH - 1),
                                skip_group_check=True,
                            )
                    nc.scalar.activation(
                        sqr[:, 2 * mp:2 * mp + 2, cur],
                        pss[mp].rearrange("p (g b) -> p g b", g=2),
                        TANH,
                    )

        # ---- output projection: outT = W_out @ hs^T + b_out ----
        with tc.tile_pool(name="psum_o", bufs=4, space="PSUM") as psum_o, \
                tc.tile_pool(name="ostg", bufs=4) as ostg:
            for mo in range(MO):
                for c in range(nch):
                    ps = psum_o.tile([128, CH], F32, name="ps_o")
                    for k in range(KH):
                        nc.tensor.matmul(
                            ps,
                            lhsT=wout_sb[k][:, mo * 128:(mo + 1) * 128],
                            rhs=sq[k][:, c * CH:(c + 1) * CH],
                            start=(k == 0),
                            stop=(k == KH - 1),
                        )
                    st = ostg.tile([128, CH], F32, name="ostg")
                    nc.vector.tensor_scalar_add(st, ps, bo[:, mo:mo + 1])
                    nc.sync.dma_start(
                        out=outT[mo * 128:(mo + 1) * 128, c * CH:(c + 1) * CH],
                        in_=st,
                    )
            for k in range(KH):
                nc.sync.dma_start(
                    out=hT_last[k * 128:(k + 1) * 128, :],
                    in_=sq[k][:, (seq - 1) * BC:seq * BC],
                )
    nc.compile()
    return nc


def make_in_maps(x, W_xh, b_xh, W_hh, b_hh, W_out, b_out, seq: int = SEQ):
    """Host-side shard + layout prep. Returns one input dict per core."""
    x = np.asarray(x, np.float32)
    w_xhT = np.ascontiguousarray(np.asarray(W_xh, np.float32).T.astype(np.float16))
    w_hhT = np.ascontiguousarray(np.asarray(W_hh, np.float32).T.astype(np.float16))
    w_outT = np.ascontiguousarray(np.asarray(W_out, np.float32).T.astype(np.float16))
    bias_h = np.ascontiguousarray(
        (np.asarray(b_xh, np.float32) + np.asarray(b_hh, np.float32))
        .reshape(KH, 128).T
    )
    bias_o = np.ascontiguousarray(np.asarray(b_out, np.float32).reshape(MO, 128).T)

    in_maps = []
    for c in range(NCORES):
        xs = x[:seq, c * BC:(c + 1) * BC, :]          # (seq, BC, IN)
        xTc = np.ascontiguousarray(
            xs.transpose(2, 0, 1).reshape(IN, seq * BC).astype(np.float16)
        )
        in_maps.append({
            "ident_dram": np.eye(128, dtype=np.float16),
            "xT": xTc,
            "w_xhT": w_xhT,
            "w_hhT": w_hhT,
            "w_outT": w_outT,
            "bias_h": bias_h,
            "bias_o": bias_o,
        })
    return in_maps


def assemble_output(results, seq: int = SEQ):
    outputs = np.empty((seq, B, OUT), np.float32)
    h_last = np.empty((B, HID), np.float32)
    for c in range(NCORES):
        oT = np.asarray(results[c]["outT"])           # (OUT, seq*BC) f32
        outputs[:, c * BC:(c + 1) * BC, :] = (
            oT.reshape(OUT, seq, BC).transpose(1, 2, 0)
        )
        h_last[c * BC:(c + 1) * BC, :] = (
            np.asarray(results[c]["hT_last"]).astype(np.float32).T
        )
    return outputs, h_last


def kernel(x, W_xh, b_xh, W_hh, b_hh, W_out, b_out):
    from concourse.bass_utils import run_bass_kernel_spmd

    nc = build_nc(SEQ)
    in_maps = make_in_maps(x, W_xh, b_xh, W_hh, b_hh, W_out, b_out, SEQ)
    res = run_bass_kernel_spmd(nc, in_maps, list(range(NCORES)))
    return assemble_output(res.results, SEQ)
